# revision 57
# baseline (speedup 1.0000x reference)
"""Jamba sparse-MoE block on 8 Trainium2 NeuronCores.

Strategy: tensor-parallel ffn (F/8 per core), host dispatch
--------
- Routing (router matmul + softmax + top-2) is computed with jax on the host
  using the exact op sequence of the reference so expert selection matches
  bit-for-bit (one token in the dataset has a top2/top3 probability gap of
  ~5e-7; any rounding difference there would flip its expert assignment).
- Every expert's FFN dim is split 8 ways (F=4096 -> 8x512); core k holds the
  k-th F-slice of ALL experts and processes the whole expert-sorted token
  stream. Per-core work is exactly sum_e ceil(L_e/128)*128 / 8 token-slots —
  the global load-balance floor: no core is pinned by the heaviest expert.
  The 8 partial outputs per token are summed on the host scatter-add.
- All matmul operands are bf16 (same PE rate as float32r on TRN2 — 1 row/cyc
  — but half the DMA bytes and no >=256 free-dim constraint). PSUM fp32.
- Each expert's token range is one phase-A/phase-B group (~2k tokens).
  Phase A (hid = silu(x@gw.T) * (x@uw.T)) keeps hid in SBUF as bf16 — no
  DRAM round-trip — and phase B (y = wt * (hid.T @ dw.T)) immediately
  consumes it. x and gate/up weights stream one group ahead; down weights
  load once, early, and stay resident. Phase B's scale runs on the DVE and
  stores issue from SP, keeping every engine's issue path under the PE rate.
- The cost model serializes all DMA on one device, so every input load is
  emitted on the sync queue in consumption-deadline order; y stores use the
  scalar queue.
"""

import math
import numpy as np
from contextlib import ExitStack

B, S, H, F, E, TOP_K = 4, 2048, 1024, 4096, 8, 2
T = B * S
N_CORES = 8
P = 128
HC = H // P  # 8 h-chunks
F8 = F // N_CORES  # 512 ffn rows per core per expert
SFB = F8 // P  # 4 f-blocks per expert segment
FB = E * SFB  # 32 f-blocks held per core
GSZ = 2176  # target tokens per phase-A/phase-B group (one expert segment)


def _token_tiles(g):
    """512-token phase-A tiles covering a group of g tokens (g % 128 == 0)."""
    tiles = [512] * (g // 512)
    if g % 512:
        tiles.append(g % 512)
    return tiles


def _split_groups(Cs):
    """Split a segment of Cs tokens into near-equal 128-multiple groups of at
    most GSZ+128 (one group per expert segment when it fits): fewer, larger
    groups mean fewer phase transitions (each PE idle gap costs ~3us of
    p-state ramp) while per-fb PE work stays far above the per-fb weight DMA
    time so the gate/up stream never starves the PE."""
    nt = Cs // P
    n = max(1, -(-nt // (GSZ // P + 1)))
    out, t0 = [], 0
    for i in range(n):
        take = (nt * (i + 1) // n - nt * i // n) * P
        if take:
            out.append((t0, take))
            t0 += take
    return out


_PROGRAM_CACHE = {}


def _build_program(caps, loads, H_=H, F_=F, act="Silu"):
    """SPMD program: one F/8-slice segment per expert, caps[e] token slots of
    which only loads[e] are real. Phase A (cost ~ token count) tiles over the
    exact loads; phase B keeps the 128-slot grid (its cost is per H-column,
    independent of token-partition occupancy). Padded slots' hid is never
    written — their phase-B output is garbage scaled by wt=0, and the host
    drops those rows anyway."""
    key = (tuple(caps) + tuple(loads), H_, F_, act)
    if key in _PROGRAM_CACHE:
        return _PROGRAM_CACHE[key]
    import concourse.bacc as bacc
    import concourse.mybir as mybir
    import concourse.tile as tile

    HC = H_ // P
    f32 = mybir.dt.float32
    bf16 = mybir.dt.bfloat16
    AF = mybir.ActivationFunctionType
    C = sum(caps)
    NT128 = C // P

    # (token_offset, group_len, fb_lo, real_len): expert e's F-slice occupies
    # f-blocks 4e..4e+4 and token slots [sum(caps[:e]), sum(caps[:e+1]));
    # only the first loads[e] slots hold real tokens.
    groups = []
    base = 0
    for e, Ce in enumerate(caps):
        for lt, g in _split_groups(Ce):
            lr = max(0, min(g, loads[e] - lt))
            groups.append((base + lt, g, SFB * e, lr))
        base += Ce

    nc = bacc.Bacc("TRN2", target_bir_lowering=False, debug=False, num_devices=N_CORES)

    x_d = nc.dram_tensor("x", [P, HC, C], bf16, kind="ExternalInput")
    gw_d = nc.dram_tensor("gw", [FB, P, HC, P], bf16, kind="ExternalInput")
    uw_d = nc.dram_tensor("uw", [FB, P, HC, P], bf16, kind="ExternalInput")
    dw_d = nc.dram_tensor("dw", [P, FB, H_], bf16, kind="ExternalInput")
    wt_d = nc.dram_tensor("wt", [NT128, P], f32, kind="ExternalInput")
    y_d = nc.dram_tensor("y", [NT128, P, H_], bf16, kind="ExternalOutput")

    hid_max = max(g for _, g, _, _ in groups)

    with tile.TileContext(nc) as tc:
        with ExitStack() as ctx:
            wtpool = ctx.enter_context(tc.tile_pool(name="wtp", bufs=1))
            xpool = ctx.enter_context(tc.tile_pool(name="xp", bufs=2))
            dwpool = ctx.enter_context(tc.tile_pool(name="dwp", bufs=1))
            gwpool = ctx.enter_context(tc.tile_pool(name="gwp", bufs=4))
            uwpool = ctx.enter_context(tc.tile_pool(name="uwp", bufs=4))
            sgpool = ctx.enter_context(tc.tile_pool(name="sgp", bufs=2))
            hidpool = ctx.enter_context(tc.tile_pool(name="hidp", bufs=1))
            ypool = ctx.enter_context(tc.tile_pool(name="yp", bufs=4))
            psg = ctx.enter_context(tc.tile_pool(name="psg", bufs=2, space="PSUM"))
            psu = ctx.enter_context(tc.tile_pool(name="psu", bufs=2, space="PSUM"))
            psy = ctx.enter_context(tc.tile_pool(name="psy", bufs=4, space="PSUM"))

            dw_t = dwpool.tile([P, FB, H_], bf16)
            wt_t = wtpool.tile([P, NT128], f32)

            # Per-group x tiles, loaded one group ahead. x_tiles[gi] is
            # created during group gi-1's phase A (gi=0 upfront).
            x_tiles = [None] * len(groups)

            def load_x(gi, c_lo, c_hi, hc_step=HC):
                # only the real tokens [0, lr) are loaded — padded columns
                # are never read by phase A
                t0, g, _, lr = groups[gi]
                if x_tiles[gi] is None:
                    x_tiles[gi] = xpool.tile([P, HC, hid_max], bf16, name="x_t")
                xt = x_tiles[gi]
                for c0 in range(c_lo, min(c_hi, lr), 512):
                    cn = min(512, lr - c0)
                    for hc in range(0, HC, hc_step):
                        nc.sync.dma_start(
                            xt[:, hc : hc + hc_step, c0 : c0 + cn],
                            x_d.ap()[:, hc : hc + hc_step, t0 + c0 : t0 + c0 + cn],
                        )

            for gi, (t0, g, fb_lo, lr) in enumerate(groups):
                # ---- Phase A: hid[f, t] = silu(g) * u, bf16 in SBUF ----
                hid_t = hidpool.tile([P, SFB, hid_max], bf16, name="hid_t")
                first_of_expert = gi == 0 or groups[gi - 1][2] != fb_lo
                for fbi in range(SFB):
                    fb = fb_lo + fbi
                    gw_t = gwpool.tile([P, HC, P], bf16, name="gw_t")
                    if gi == 0 and fbi == 0:
                        # Startup critical path: the first Ldweights only
                        # needs gw0's hc0 slice, and the first matmul only
                        # the first 2-hc x piece — land those two small
                        # transfers first, then the remainders.
                        xt = xpool.tile([P, HC, hid_max], bf16, name="x_t")
                        x_tiles[0] = xt
                        nc.sync.dma_start(gw_t[:, 0:2, :], gw_d.ap()[fb][:, 0:2, :])
                        nc.sync.dma_start(xt[:, 0:2, 0:512], x_d.ap()[:, 0:2, 0:512])
                        nc.sync.dma_start(gw_t[:, 2:, :], gw_d.ap()[fb][:, 2:, :])
                        for hc in range(2, HC, 2):
                            nc.sync.dma_start(
                                xt[:, hc : hc + 2, 0:512], x_d.ap()[:, hc : hc + 2, 0:512]
                            )
                    else:
                        nc.sync.dma_start(gw_t[:], gw_d.ap()[fb])
                    uw_t = uwpool.tile([P, HC, P], bf16, name="uw_t")
                    nc.sync.dma_start(uw_t[:], uw_d.ap()[fb])
                    if gi == 0 and fbi == 0:
                        load_x(0, 512, lr, hc_step=4)
                    # down weights for expert e, during its first group's
                    # phase A — BEFORE the next group's x on the queue: dw is
                    # needed at this group's phase B, x only a phase later.
                    # (x ahead of dw delays dw's transfer into phase B, which
                    # then delays the y-stores queued behind it and stalls
                    # the PE through the y/PSUM buffer rotation.)
                    if fbi == 1 and first_of_expert:
                        # per-f-block pieces (~0.7us each, not one 2.9us
                        # transfer) so phase-B y-stores sharing the DMA
                        # device aren't blocked behind a monolithic load
                        for q in range(SFB):
                            nc.sync.dma_start(
                                dw_t[:, fb_lo + q, :], dw_d.ap()[:, fb_lo + q, :]
                            )
                    # next group's tokens stream during this group's phase A
                    if fbi == 2 and gi + 1 < len(groups):
                        load_x(gi + 1, 0, groups[gi + 1][3], hc_step=4)
                    if gi == 0 and fbi == 3:
                        nc.sync.dma_start(wt_t[:], wt_d.ap().rearrange("n p -> p n"))
                    x_t = x_tiles[gi]
                    tt = 0
                    for nt in _token_tiles(lr):
                        ps_g = psg.tile([P, 512], f32, name="ps_g")[:, :nt]
                        ps_u = psu.tile([P, 512], f32, name="ps_u")[:, :nt]
                        chains = [(ps_g, gw_t, hc) for hc in range(HC)] + [
                            (ps_u, uw_t, hc) for hc in range(HC)
                        ]
                        for ps, wt_, hc in chains:
                            nc.tensor.matmul(
                                ps,
                                wt_[:, hc, :],
                                x_t[:, hc, tt : tt + nt],
                                start=(hc == 0),
                                stop=(hc == HC - 1),
                            )
                        sg = sgpool.tile([P, 512], f32, name="sg")[:, :nt]
                        nc.scalar.activation(sg, ps_g, getattr(AF, act))
                        nc.vector.tensor_mul(hid_t[:, fbi, tt : tt + nt], sg, ps_u)
                        tt += nt

                # ---- Phase B: y[t, :] = wt[t] * (hid[:, t].T @ dw.T) ----
                # Phase B has only ~1.7us of PE work per 128-token sub-tile,
                # so the scale runs on the (otherwise idle) DVE and the bf16
                # store issues from the SP queue — keeping the Act engine and
                # its HWDGE issue path out of phase B entirely. Separate
                # accumulation chains per H-half so half 0's scale+store
                # overlaps half 1's matmuls.
                last_group = gi == len(groups) - 1
                for sub in range(g // P):
                    tt128 = t0 // P + sub
                    # On the program's very last sub-tile, use four H-quarter
                    # chains so only a quarter's scale+store remains after
                    # the final matmul (shrinks the end-of-program drain).
                    nslc = 4 if last_group and sub == g // P - 1 else 2
                    wslc = H_ // nslc
                    y_sb = ypool.tile([P, H_], bf16, name="y_sb")
                    for hh in range(nslc):
                        ps_y = psy.tile([P, 512], f32, name="ps_y")[:, :wslc]
                        for fbi in range(SFB):
                            nc.tensor.matmul(
                                ps_y,
                                hid_t[:, fbi, sub * P : (sub + 1) * P],
                                dw_t[:, fb_lo + fbi, hh * wslc : (hh + 1) * wslc],
                                start=(fbi == 0),
                                stop=(fbi == SFB - 1),
                            )
                        nc.vector.tensor_scalar_mul(
                            y_sb[:, hh * wslc : (hh + 1) * wslc],
                            ps_y,
                            wt_t[:, tt128 : tt128 + 1],
                        )
                        if nslc == 4 and hh % 2 == 1:
                            # last sub-tile: two half-stores (fewer HWDGE
                            # issue holds before the final semaphore) while
                            # the quarter chains keep the PE tail fine
                            nc.sync.dma_start(
                                y_d.ap()[tt128][:, (hh - 1) * wslc : (hh + 1) * wslc],
                                y_sb[:, (hh - 1) * wslc : (hh + 1) * wslc],
                            )
                    if nslc == 2:
                        nc.sync.dma_start(y_d.ap()[tt128], y_sb[:])
    nc.compile()
    _PROGRAM_CACHE[key] = nc
    return nc


def _routing(hidden_states, router_w):
    """Replicate the reference's routing ops exactly (same jax ops, default
    platform) so top-2 selection matches bit-for-bit."""
    import jax
    import jax.numpy as jnp

    x = jnp.asarray(hidden_states).reshape(-1, H)
    router_logits = x @ jnp.asarray(router_w).T
    routing_weights = jax.nn.softmax(router_logits.astype(jnp.float32), axis=-1)
    top_k_weights, top_k_index = jax.lax.top_k(routing_weights, TOP_K)
    return np.asarray(top_k_index), np.asarray(top_k_weights, dtype=np.float32)


def kernel(hidden_states, router_w, gate_w, up_w, down_w):
    import ml_dtypes
    from concourse.bass_utils import run_bass_kernel_spmd

    bf16 = ml_dtypes.bfloat16
    hidden_states = np.asarray(hidden_states, dtype=np.float32)
    router_w = np.asarray(router_w, dtype=np.float32)
    gate_w = np.asarray(gate_w, dtype=np.float32)
    up_w = np.asarray(up_w, dtype=np.float32)
    down_w = np.asarray(down_w, dtype=np.float32)

    tki, tkw = _routing(hidden_states, router_w)
    xf = hidden_states.reshape(T, H).astype(bf16)

    idx_list, w_list = [], []
    for e in range(E):
        sel = tki == e  # [T, 2]
        tok = sel.any(axis=1)
        idx = np.nonzero(tok)[0]
        w = np.where(sel[:, 0], tkw[:, 0], tkw[:, 1])[idx]
        idx_list.append(idx)
        w_list.append(w.astype(np.float32))

    caps = tuple(
        max(256, int(math.ceil(len(idx_list[e]) / 128.0)) * 128) for e in range(E)
    )
    C = sum(caps)
    NT128 = C // P

    nc = _build_program(caps, tuple(len(idx_list[e]) for e in range(E)))

    # x / wt: the global expert-sorted padded token stream, same on all cores
    xg = np.zeros((C, H), bf16)
    wp = np.zeros((C,), np.float32)
    base = 0
    for e in range(E):
        ne = len(idx_list[e])
        xg[base : base + ne] = xf[idx_list[e]]
        wp[base : base + ne] = w_list[e]
        base += caps[e]
    x_in = np.ascontiguousarray(xg.T.reshape(HC, P, C).transpose(1, 0, 2))
    wt_in = np.ascontiguousarray(wp.reshape(NT128, P))

    gwb = gate_w.astype(bf16)
    uwb = up_w.astype(bf16)
    dwb = np.ascontiguousarray(down_w.transpose(0, 2, 1)).astype(bf16)  # [E, F, H]

    in_maps = []
    for k in range(N_CORES):
        fs = slice(k * F8, (k + 1) * F8)
        # [E, F8, H] -> blocks of 128 f-rows, partition dim = h-within-chunk
        gslc = gwb[:, fs].reshape(FB, P, HC, P).transpose(0, 3, 2, 1)
        uslc = uwb[:, fs].reshape(FB, P, HC, P).transpose(0, 3, 2, 1)
        dslc = dwb[:, fs].reshape(FB, P, H).transpose(1, 0, 2)
        in_maps.append(
            {
                "x": x_in,
                "gw": np.ascontiguousarray(gslc),
                "uw": np.ascontiguousarray(uslc),
                "dw": np.ascontiguousarray(dslc),
                "wt": wt_in,
            }
        )

    res = run_bass_kernel_spmd(nc, in_maps, core_ids=list(range(N_CORES)))

    ysum = res.results[0]["y"].reshape(C, H).astype(np.float64)
    for k in range(1, N_CORES):
        ysum += res.results[k]["y"].reshape(C, H)
    ysum = ysum.astype(np.float32)

    out = np.zeros((T, H), np.float32)
    base = 0
    for e in range(E):
        ne = len(idx_list[e])
        out[idx_list[e]] += ysum[base : base + ne]
        base += caps[e]
    return out.reshape(B, S, H)
